# revision 9
# baseline (speedup 1.0000x reference)
"""GRU observation-cell kernel for Trainium2 (8 NeuronCores).

Reference computation (torch GRUCell gate order r,z,n):
    x = X_obs.reshape(M, 128); hs = h[i_obs]
    gi = x@W_ih.T + b_ih; gh = hs@W_hh.T + b_hh
    r = sigmoid(gi_r + gh_r); z = sigmoid(gi_z + gh_z)
    n = tanh(gi_n + b_in + r*(gh_n_raw + b_hn))
    h_new = n + z*(hs - n);  out = h.at[i_obs].set(h_new)

Strategy: data-parallel over the M=20000 observed rows, 2500 rows/core;
host does the (free) gather/scatter, transposes, and dtype casts.  The
device kernel is Act-engine-bound (sigmoid+tanh are Act-only, 768
activation elems/row), so everything else is arranged to hide under the
Act stream:

  - All matmuls are fp8e4 DoubleRow (0.5 PE-cycles per output column,
    256-contract per pass).  rhs tile [128, 5, ts] fp8 with slots
    [pad,x,h0,h1,pad] (pad row0 = 1.0) makes every needed contract pair
    slot-adjacent: i_n=(pad,x), rz1=(x,h0), hn=(h0,h1), rz2=(h1,pad).
    All biases ride pad-row weights, so sigmoid covers r0r1z0z1 in ONE
    Act instruction and tanh needs no bias operand either.
  - PSUM slots are padded to 512 cols so every matmul output is
    bank-aligned: ps_rz [128,4,512] + ps_hn [128,2,512] + ps_n
    [128,2,512] = exactly 8 banks, all bufs=1.
  - n-gate: b_hn is folded into ps_hn via an extra DoubleRow bias pass,
    so t1 = ps_hn * r is ONE plain DVE mult (fp8 out); the next iter PE
    runs i_n (start=True) then an identity DoubleRow pass accumulates t1
    (the zero-weight half ignores the other t1 slot).
  - ~120 tiny warm-up matmuls run while the input DMAs land so the PE is
    at full p-state when tile 0's real matmuls start.
  - Software pipeline (iter t): PE rz/hn-mm(t), in+id-mm(t-1);
    Act sigmoid_rz(t), tanh(t-1); DVE t1(t), blend(t-1); Pool d_j0(t-1);
    SP dma-out(t-1).  Col-tiles [372,500,500,500,500,128]: small edge
    tiles shorten pipeline fill/drain; drain tiles split sigmoid r/z and
    j-halves to shorten the final dependency chains.

Cost-model timing: 28633 ns vs 41682 ns for the fp32r baseline (1.46x);
rel err 7.1e-3 overall (fp8 quantization noise), vs 2e-2 gate.
"""

import numpy as np

N, H, IN2, M, NCORES = 100000, 256, 128, 20000, 8
MC = M // NCORES                      # 2500 observed rows per core
TS = [436, 500, 500, 500, 500, 64]
OFF = np.cumsum([0] + TS).tolist()    # col offsets
NRT = len(TS)

_compiled = {}


def _build_nc():
    from contextlib import ExitStack

    from concourse import bacc
    import concourse.mybir as mybir
    from concourse.tile import TileContext

    dt = mybir.dt
    f32 = dt.float32
    bf16 = dt.bfloat16
    f8 = dt.float8e4
    AF = mybir.ActivationFunctionType
    ALU = mybir.AluOpType
    DR = mybir.MatmulPerfMode.DoubleRow

    nc = bacc.Bacc(None, target_bir_lowering=False)

    # flat per-partition layouts: tile t occupies cols [5*off, 5*(off+ts))
    rhs_d = nc.dram_tensor("rhs8", [128, 5 * MC], f8, kind="ExternalInput")
    wrzb_d = nc.dram_tensor("wrzb", [128, 2, 8, 128], f8, kind="ExternalInput")
    wnb_d = nc.dram_tensor("wnb", [128, 2, 8, 128], f8, kind="ExternalInput")
    h16_d = nc.dram_tensor("h16", [128, 2 * MC], bf16, kind="ExternalInput")
    out_d = nc.dram_tensor("hout", [128, 2 * MC], bf16, kind="ExternalOutput")

    with TileContext(nc) as tc, ExitStack() as ctx:
        const = ctx.enter_context(tc.tile_pool(name="const", bufs=1))
        rhsp = ctx.enter_context(tc.tile_pool(name="rhsp", bufs=1))
        work = ctx.enter_context(tc.tile_pool(name="work", bufs=1))
        psum = ctx.enter_context(tc.tile_pool(name="psum", bufs=1, space="PSUM"))

        # --- PE warm-up: tiny matmuls on a memset tile while DMAs land ---
        warm = const.tile([128, 2, 128], f8, tag="warm")
        nc.gpsimd.memset(warm[:], 0.0)
        ps_warm = psum.tile([128, 4, 512], f32, tag="rz", name="ps_warm")
        for _ in range(120):
            nc.tensor.matmul(
                ps_warm[:, 0, 0:64], lhsT=warm[:, :, :], rhs=warm[:, :, 0:64],
                start=True, stop=True, perf_mode=DR,
            )

        # --- input DMAs (SP queue; rhs tiles all resident) ---
        rhs_t = [None] * NRT
        rhs_t[0] = rhsp.tile([128, 5, TS[0]], f8, tag="rhs0", name="rhs0")
        nc.sync.dma_start(out=rhs_t[0][:], in_=rhs_d[:, 0 : 5 * TS[0]])
        wrzb = const.tile([128, 2, 8, 128], f8, tag="wrzb")
        nc.sync.dma_start(out=wrzb[:], in_=wrzb_d[...])
        wnb = const.tile([128, 2, 8, 128], f8, tag="wnb")
        nc.sync.dma_start(out=wnb[:], in_=wnb_d[...])
        h16_t = [None] * NRT
        h16_t[0] = rhsp.tile([128, 2, TS[0]], bf16, tag="h16_0", name="h16_0")
        nc.sync.dma_start(out=h16_t[0][:], in_=h16_d[:, 0 : 2 * TS[0]])
        for t in range(1, NRT):
            rhs_t[t] = rhsp.tile([128, 5, TS[t]], f8, tag=f"rhs{t}", name=f"rhs{t}")
            nc.sync.dma_start(
                out=rhs_t[t][:], in_=rhs_d[:, 5 * OFF[t] : 5 * OFF[t + 1]]
            )
            h16_t[t] = rhsp.tile([128, 2, TS[t]], bf16, tag=f"h16_{t}",
                                 name=f"h16_{t}")
            nc.sync.dma_start(
                out=h16_t[t][:], in_=h16_d[:, 2 * OFF[t] : 2 * OFF[t + 1]]
            )

        # --- software-pipelined main loop ---
        ps_rz = [None] * NRT
        ps_hn = [None] * NRT
        ps_n = [None] * NRT
        t1b = [None] * NRT
        rz16 = [None] * NRT
        n16 = [None] * NRT
        d16 = [None] * NRT
        e16 = [None] * NRT
        ho = [None] * NRT

        for t in range(NRT + 1):
            if t < NRT:
                r = rhs_t[t]
                ts_ = TS[t]
                # PE: r,z gates — two DoubleRow passes per gate tile
                ps_rz[t] = psum.tile([128, 4, 512], f32, tag="rz", name=f"psrz{t}")
                for g in range(4):
                    nc.tensor.matmul(
                        ps_rz[t][:, g, 0:ts_], lhsT=wrzb[:, :, g, :],
                        rhs=r[:, 1:3, :],
                        start=True, stop=False, perf_mode=DR,
                    )
                    nc.tensor.matmul(
                        ps_rz[t][:, g, 0:ts_], lhsT=wrzb[:, :, 4 + g, :],
                        rhs=r[:, 3:5, :],
                        start=False, stop=True, perf_mode=DR,
                    )
                # PE: h_n gate (contract 256, one DoubleRow pass per half)
                ps_hn[t] = psum.tile([128, 2, 512], f32, tag="hn", name=f"pshn{t}")
                for j in range(2):
                    nc.tensor.matmul(
                        ps_hn[t][:, j, 0:ts_], lhsT=wnb[:, :, 2 + j, :],
                        rhs=r[:, 2:4, :],
                        start=True, stop=False, perf_mode=DR,
                    )
                    nc.tensor.matmul(
                        ps_hn[t][:, j, 0:ts_], lhsT=wnb[:, :, 6 + j, :],
                        rhs=r[:, 0:2, :],
                        start=False, stop=True, perf_mode=DR,
                    )

            if 0 <= t - 1 < NRT:
                u = t - 1
                us = TS[u]
                # PE: i_n (+b_in via pad row), then identity DoubleRow pass
                # adds t1 (the zero-weight half ignores the other t1 slot)
                ps_n[u] = psum.tile([128, 2, 512], f32, tag="n", name=f"psn{u}")
                for j in range(2):
                    nc.tensor.matmul(
                        ps_n[u][:, j, 0:us], lhsT=wnb[:, :, j, :],
                        rhs=rhs_t[u][:, 0:2, :],
                        start=True, stop=False, perf_mode=DR,
                    )
                    nc.tensor.matmul(
                        ps_n[u][:, j, 0:us], lhsT=wnb[:, :, 4 + j, :],
                        rhs=t1b[u][:, 0:2, :],
                        start=False, stop=True, perf_mode=DR,
                    )

            if t < NRT:
                ts_ = TS[t]
                # Act: sigmoid over r/z gate tiles; drain tiles split r first
                # so the t1 chain starts one sigmoid earlier
                rz16[t] = work.tile([128, 4, ts_], bf16, tag=f"rz16_{t}", bufs=1,
                                    name=f"rz16_{t}")
                if t >= NRT - 2:
                    nc.scalar.activation(
                        out=rz16[t][:, 0:2, :], in_=ps_rz[t][:, 0:2, 0:ts_],
                        func=AF.Sigmoid,
                    )
                    nc.scalar.activation(
                        out=rz16[t][:, 2:4, :], in_=ps_rz[t][:, 2:4, 0:ts_],
                        func=AF.Sigmoid,
                    )
                else:
                    nc.scalar.activation(
                        out=rz16[t][:], in_=ps_rz[t][:, :, 0:ts_], func=AF.Sigmoid
                    )

            if 0 <= t - 1 < NRT:
                u = t - 1
                us = TS[u]
                # Act: tanh of previous tile's n gate (software-pipelined)
                n16[u] = work.tile([128, 2, us], bf16, tag=f"n16_{u}", bufs=1,
                                   name=f"n16_{u}")
                if u >= NRT - 2:
                    for j in range(2):
                        nc.scalar.activation(
                            out=n16[u][:, j, :], in_=ps_n[u][:, j, 0:us],
                            func=AF.Tanh,
                        )
                else:
                    nc.scalar.activation(
                        out=n16[u][:], in_=ps_n[u][:, :, 0:us], func=AF.Tanh
                    )

            if t < NRT:
                ts_ = TS[t]
                # DVE: t1 = (ps_hn + b_hn) * r  -> SBUF fp8 (id-matmul rhs)
                t1b[t] = work.tile([128, 2, ts_], f8, tag=f"t1b_{t}", bufs=1,
                                   name=f"t1b{t}")
                with tc.high_priority():
                    nc.vector.tensor_tensor(
                        out=t1b[t][:], in0=ps_hn[t][:, :, 0:ts_],
                        in1=rz16[t][:, 0:2, :], op=ALU.mult,
                    )

            if 0 <= t - 1 < NRT:
                u = t - 1
                us = TS[u]
                # blend, j-split: d_j0 on Pool in parallel with d_j1 on DVE,
                # then e/add on DVE, store via SP — all one iter behind
                d16[u] = work.tile([128, 2, us], bf16, tag=f"d16_{u}", bufs=1,
                                   name=f"d16_{u}")
                e16[u] = work.tile([128, 2, us], bf16, tag=f"e16_{u}", bufs=1,
                                   name=f"e16_{u}")
                ho[u] = work.tile([128, 2, us], bf16, tag=f"ho_{u}", bufs=1,
                                  name=f"ho{u}")
                for j in range(2):
                    deng = nc.gpsimd if j == 0 and u < NRT - 1 else nc.vector
                    deng.tensor_tensor(
                        out=d16[u][:, j, :], in0=h16_t[u][:, j, :],
                        in1=n16[u][:, j, :], op=ALU.subtract,
                    )
                for j in range(2):
                    nc.vector.tensor_tensor(
                        out=e16[u][:, j, :], in0=rz16[u][:, 2 + j, :],
                        in1=d16[u][:, j, :], op=ALU.mult,
                    )
                    nc.vector.tensor_tensor(
                        out=ho[u][:, j, :], in0=n16[u][:, j, :],
                        in1=e16[u][:, j, :], op=ALU.add,
                    )
                nc.sync.dma_start(
                    out=out_d[:, 2 * OFF[u] : 2 * OFF[u + 1]], in_=ho[u][:]
                )

    nc.compile()
    return nc


def _get_nc():
    if "nc" not in _compiled:
        _compiled["nc"] = _build_nc()
    return _compiled["nc"]


def _make_in_maps(h, X_obs, i_obs, W_ih, W_hh, b_ih, b_hh):
    import ml_dtypes

    f = np.float32
    f8 = ml_dtypes.float8_e4m3

    x2 = np.asarray(X_obs, f).reshape(M, IN2)
    hs = np.asarray(h, f)[np.asarray(i_obs)]
    W_ih = np.asarray(W_ih, f)
    W_hh = np.asarray(W_hh, f)
    b_ih = np.asarray(b_ih, f)
    b_hh = np.asarray(b_hh, f)
    brz = b_ih[: 2 * H] + b_hh[: 2 * H]          # [512]
    bin_ = b_ih[2 * H :]                          # [256]
    bhn = b_hh[2 * H :]                           # [256]

    # r/z weight blob: [p, half, 0..3]=pass1 (x,h0), [p, half, 4..7]=pass2
    wrzb = np.zeros((128, 2, 8, 128), f)
    for g in range(4):
        rows = slice(g * 128, (g + 1) * 128)
        wrzb[:, 0, g, :] = W_ih[rows, :].T            # x half
        wrzb[:, 1, g, :] = W_hh[rows, 0:128].T        # h0 half
        wrzb[:, 0, 4 + g, :] = W_hh[rows, 128:256].T  # h1 half
        wrzb[0, 1, 4 + g, :] = brz[rows]              # bias row (pad half)
    # n-path blob: [0..1]=i_n (pad,x), [2..3]=h_n (h0,h1), [4..5]=identity
    wnb = np.zeros((128, 2, 8, 128), f)
    for j in range(2):
        rows = slice(2 * H + j * 128, 2 * H + (j + 1) * 128)
        wnb[0, 0, j, :] = bin_[j * 128 : (j + 1) * 128]  # bias row (pad half)
        wnb[:, 1, j, :] = W_ih[rows, :].T                # x half
        wnb[:, 0, 2 + j, :] = W_hh[rows, 0:128].T
        wnb[:, 1, 2 + j, :] = W_hh[rows, 128:256].T
        wnb[:, j, 4 + j, :] = np.eye(128, dtype=f)       # id for t1_j slot
        wnb[0, 0, 6 + j, :] = bhn[j * 128 : (j + 1) * 128]  # b_hn bias row
    wrzb = wrzb.astype(f8)
    wnb = wnb.astype(f8)

    in_maps = []
    for c in range(NCORES):
        rows0 = c * MC
        rhs8 = np.zeros((128, 5 * MC), f)
        h16 = np.zeros((128, 2 * MC), f)
        for t in range(NRT):
            rs = slice(rows0 + OFF[t], rows0 + OFF[t + 1])
            blk = np.zeros((128, 5, TS[t]), f)
            blk[0, 0, :] = 1.0                    # pad (bias rows)
            blk[:, 1, :] = x2[rs, :].T
            blk[:, 2, :] = hs[rs, 0:128].T
            blk[:, 3, :] = hs[rs, 128:256].T
            blk[0, 4, :] = 1.0                    # pad (bias rows)
            rhs8[:, 5 * OFF[t] : 5 * OFF[t + 1]] = blk.reshape(128, -1)
            hblk = np.stack([hs[rs, 0:128].T, hs[rs, 128:256].T], axis=1)
            h16[:, 2 * OFF[t] : 2 * OFF[t + 1]] = hblk.reshape(128, -1)
        in_maps.append(
            {
                "rhs8": rhs8.astype(f8),
                "h16": h16.astype(ml_dtypes.bfloat16),
                "wrzb": wrzb,
                "wnb": wnb,
            }
        )
    return in_maps


def run_on_device(h, X_obs, i_obs, W_ih, W_hh, b_ih, b_hh, **run_kwargs):
    """Returns (h_new [M,H] fp32, BassKernelResults)."""
    from concourse.bass_utils import run_bass_kernel_spmd

    in_maps = _make_in_maps(h, X_obs, i_obs, W_ih, W_hh, b_ih, b_hh)
    res = run_bass_kernel_spmd(_get_nc(), in_maps, list(range(NCORES)), **run_kwargs)
    parts = []
    for r in res.results:
        ho = np.asarray(r["hout"], dtype=np.float32)      # [128, 2*MC]
        hr = np.zeros((MC, H), np.float32)
        for t in range(NRT):
            blk = ho[:, 2 * OFF[t] : 2 * OFF[t + 1]].reshape(128, 2, TS[t])
            hr[OFF[t] : OFF[t + 1], 0:128] = blk[:, 0, :].T
            hr[OFF[t] : OFF[t + 1], 128:256] = blk[:, 1, :].T
        parts.append(hr)
    h_new = np.concatenate(parts, axis=0)
    return h_new, res


def kernel(h, X_obs, i_obs, W_ih, W_hh, b_ih, b_hh):
    h = np.asarray(h, np.float32)
    i_obs = np.asarray(i_obs)
    h_new, _ = run_on_device(h, X_obs, i_obs, W_ih, W_hh, b_ih, b_hh)
    out = h.copy()
    out[i_obs] = h_new
    return out


# revision 10
# speedup vs baseline: 1.0093x; 1.0093x over previous
"""GRU observation-cell kernel for Trainium2 (8 NeuronCores).

Reference computation (torch GRUCell gate order r,z,n):
    x = X_obs.reshape(M, 128); hs = h[i_obs]
    gi = x@W_ih.T + b_ih; gh = hs@W_hh.T + b_hh
    r = sigmoid(gi_r + gh_r); z = sigmoid(gi_z + gh_z)
    n = tanh(gi_n + b_in + r*(gh_n_raw + b_hn))
    h_new = n + z*(hs - n);  out = h.at[i_obs].set(h_new)

Strategy: data-parallel over the M=20000 observed rows, 2500 rows/core;
host does the (free) gather/scatter, transposes, and dtype casts.  The
device kernel is Act-engine-bound (sigmoid+tanh are Act-only, 768
activation elems/row), so everything else is arranged to hide under the
Act stream:

  - All matmuls are fp8e4 DoubleRow (0.5 PE-cycles per output column,
    256-contract per pass).  rhs tile [128, 5, ts] fp8 with slots
    [pad,x,h0,h1,pad] (pad row0 = 1.0) makes every needed contract pair
    slot-adjacent: i_n=(pad,x), rz1=(x,h0), hn=(h0,h1), rz2=(h1,pad).
    All biases ride pad-row weights, so sigmoid covers r0r1z0z1 in ONE
    Act instruction and tanh needs no bias operand either.
  - PSUM slots are padded to 512 cols so every matmul output is
    bank-aligned: ps_rz [128,4,512] + ps_hn [128,2,512] + ps_n
    [128,2,512] = exactly 8 banks, all bufs=1.
  - n-gate: b_hn is folded into ps_hn via an extra DoubleRow bias pass,
    so t1 = ps_hn * r is ONE plain DVE mult (fp8 out); the next iter PE
    runs i_n (start=True) then an identity DoubleRow pass accumulates t1
    (the zero-weight half ignores the other t1 slot).
  - ~120 tiny warm-up matmuls run while the input DMAs land so the PE is
    at full p-state when tile 0's real matmuls start.
  - Software pipeline (iter t): PE rz/hn-mm(t), in+id-mm(t-1);
    Act sigmoid_rz(t), tanh(t-1); DVE t1(t), blend(t-1); Pool d_j0(t-1);
    SP dma-out(t-1).  Col-tiles [372,500,500,500,500,128]: small edge
    tiles shorten pipeline fill/drain; drain tiles split sigmoid r/z and
    j-halves to shorten the final dependency chains.

Cost-model timing: 28633 ns vs 41682 ns for the fp32r baseline (1.46x);
rel err 7.1e-3 overall (fp8 quantization noise), vs 2e-2 gate.
"""

import numpy as np

N, H, IN2, M, NCORES = 100000, 256, 128, 20000, 8
MC = M // NCORES                      # 2500 observed rows per core
TS = [372, 500, 500, 500, 500, 128]
OFF = np.cumsum([0] + TS).tolist()    # col offsets
NRT = len(TS)

_compiled = {}


def _build_nc():
    from contextlib import ExitStack

    from concourse import bacc
    import concourse.mybir as mybir
    from concourse.tile import TileContext

    dt = mybir.dt
    f32 = dt.float32
    bf16 = dt.bfloat16
    f8 = dt.float8e4
    AF = mybir.ActivationFunctionType
    ALU = mybir.AluOpType
    DR = mybir.MatmulPerfMode.DoubleRow

    nc = bacc.Bacc(None, target_bir_lowering=False)

    # flat per-partition layouts: tile t occupies cols [5*off, 5*(off+ts))
    rhs_d = nc.dram_tensor("rhs8", [128, 5 * MC], f8, kind="ExternalInput")
    wrzb_d = nc.dram_tensor("wrzb", [128, 2, 8, 128], f8, kind="ExternalInput")
    wnb_d = nc.dram_tensor("wnb", [128, 2, 8, 128], f8, kind="ExternalInput")
    h16_d = nc.dram_tensor("h16", [128, 2 * MC], bf16, kind="ExternalInput")
    out_d = nc.dram_tensor("hout", [128, 2 * MC], bf16, kind="ExternalOutput")

    with TileContext(nc) as tc, ExitStack() as ctx:
        const = ctx.enter_context(tc.tile_pool(name="const", bufs=1))
        rhsp = ctx.enter_context(tc.tile_pool(name="rhsp", bufs=1))
        work = ctx.enter_context(tc.tile_pool(name="work", bufs=1))
        psum = ctx.enter_context(tc.tile_pool(name="psum", bufs=1, space="PSUM"))

        # --- PE warm-up: tiny matmuls on a memset tile while DMAs land ---
        warm = const.tile([128, 2, 128], f8, tag="warm")
        nc.gpsimd.memset(warm[:], 0.0)
        ps_warm = psum.tile([128, 4, 512], f32, tag="rz", name="ps_warm")
        for _ in range(112):
            nc.tensor.matmul(
                ps_warm[:, 0, 0:64], lhsT=warm[:, :, :], rhs=warm[:, :, 0:64],
                start=True, stop=True, perf_mode=DR,
            )

        # --- input DMAs (SP queue; rhs tiles all resident) ---
        rhs_t = [None] * NRT
        rhs_t[0] = rhsp.tile([128, 5, TS[0]], f8, tag="rhs0", name="rhs0")
        nc.sync.dma_start(out=rhs_t[0][:], in_=rhs_d[:, 0 : 5 * TS[0]])
        wrzb = const.tile([128, 2, 8, 128], f8, tag="wrzb")
        nc.sync.dma_start(out=wrzb[:], in_=wrzb_d[...])
        wnb = const.tile([128, 2, 8, 128], f8, tag="wnb")
        nc.sync.dma_start(out=wnb[:], in_=wnb_d[...])
        h16_t = [None] * NRT
        h16_t[0] = rhsp.tile([128, 2, TS[0]], bf16, tag="h16_0", name="h16_0")
        nc.sync.dma_start(out=h16_t[0][:], in_=h16_d[:, 0 : 2 * TS[0]])
        for t in range(1, NRT):
            rhs_t[t] = rhsp.tile([128, 5, TS[t]], f8, tag=f"rhs{t}", name=f"rhs{t}")
            nc.sync.dma_start(
                out=rhs_t[t][:], in_=rhs_d[:, 5 * OFF[t] : 5 * OFF[t + 1]]
            )
            h16_t[t] = rhsp.tile([128, 2, TS[t]], bf16, tag=f"h16_{t}",
                                 name=f"h16_{t}")
            nc.sync.dma_start(
                out=h16_t[t][:], in_=h16_d[:, 2 * OFF[t] : 2 * OFF[t + 1]]
            )

        # --- software-pipelined main loop ---
        ps_rz = [None] * NRT
        ps_hn = [None] * NRT
        ps_n = [None] * NRT
        t1b = [None] * NRT
        rz16 = [None] * NRT
        n16 = [None] * NRT
        d16 = [None] * NRT
        e16 = [None] * NRT
        ho = [None] * NRT

        for t in range(NRT + 1):
            if t < NRT:
                r = rhs_t[t]
                ts_ = TS[t]
                # PE: r,z gates — two DoubleRow passes per gate tile
                ps_rz[t] = psum.tile([128, 4, 512], f32, tag="rz", name=f"psrz{t}")
                for g in range(4):
                    nc.tensor.matmul(
                        ps_rz[t][:, g, 0:ts_], lhsT=wrzb[:, :, g, :],
                        rhs=r[:, 1:3, :],
                        start=True, stop=False, perf_mode=DR,
                    )
                    nc.tensor.matmul(
                        ps_rz[t][:, g, 0:ts_], lhsT=wrzb[:, :, 4 + g, :],
                        rhs=r[:, 3:5, :],
                        start=False, stop=True, perf_mode=DR,
                    )
                # PE: h_n gate (contract 256, one DoubleRow pass per half)
                ps_hn[t] = psum.tile([128, 2, 512], f32, tag="hn", name=f"pshn{t}")
                for j in range(2):
                    nc.tensor.matmul(
                        ps_hn[t][:, j, 0:ts_], lhsT=wnb[:, :, 2 + j, :],
                        rhs=r[:, 2:4, :],
                        start=True, stop=False, perf_mode=DR,
                    )
                    nc.tensor.matmul(
                        ps_hn[t][:, j, 0:ts_], lhsT=wnb[:, :, 6 + j, :],
                        rhs=r[:, 0:2, :],
                        start=False, stop=True, perf_mode=DR,
                    )

            if 0 <= t - 1 < NRT:
                u = t - 1
                us = TS[u]
                # PE: i_n (+b_in via pad row), then identity DoubleRow pass
                # adds t1 (the zero-weight half ignores the other t1 slot)
                ps_n[u] = psum.tile([128, 2, 512], f32, tag="n", name=f"psn{u}")
                for j in range(2):
                    nc.tensor.matmul(
                        ps_n[u][:, j, 0:us], lhsT=wnb[:, :, j, :],
                        rhs=rhs_t[u][:, 0:2, :],
                        start=True, stop=False, perf_mode=DR,
                    )
                    nc.tensor.matmul(
                        ps_n[u][:, j, 0:us], lhsT=wnb[:, :, 4 + j, :],
                        rhs=t1b[u][:, 0:2, :],
                        start=False, stop=True, perf_mode=DR,
                    )

            if t < NRT:
                ts_ = TS[t]
                # Act: sigmoid over r/z gate tiles; drain tiles split r first
                # so the t1 chain starts one sigmoid earlier
                rz16[t] = work.tile([128, 4, ts_], bf16, tag=f"rz16_{t}", bufs=1,
                                    name=f"rz16_{t}")
                if t >= NRT - 2:
                    nc.scalar.activation(
                        out=rz16[t][:, 0:2, :], in_=ps_rz[t][:, 0:2, 0:ts_],
                        func=AF.Sigmoid,
                    )
                    nc.scalar.activation(
                        out=rz16[t][:, 2:4, :], in_=ps_rz[t][:, 2:4, 0:ts_],
                        func=AF.Sigmoid,
                    )
                else:
                    nc.scalar.activation(
                        out=rz16[t][:], in_=ps_rz[t][:, :, 0:ts_], func=AF.Sigmoid
                    )

            if 0 <= t - 1 < NRT:
                u = t - 1
                us = TS[u]
                # Act: tanh of previous tile's n gate (software-pipelined)
                n16[u] = work.tile([128, 2, us], bf16, tag=f"n16_{u}", bufs=1,
                                   name=f"n16_{u}")
                if u >= NRT - 2:
                    for j in range(2):
                        nc.scalar.activation(
                            out=n16[u][:, j, :], in_=ps_n[u][:, j, 0:us],
                            func=AF.Tanh,
                        )
                else:
                    nc.scalar.activation(
                        out=n16[u][:], in_=ps_n[u][:, :, 0:us], func=AF.Tanh
                    )

            if t < NRT:
                ts_ = TS[t]
                # DVE: t1 = (ps_hn + b_hn) * r  -> SBUF fp8 (id-matmul rhs)
                t1b[t] = work.tile([128, 2, ts_], f8, tag=f"t1b_{t}", bufs=1,
                                   name=f"t1b{t}")
                with tc.high_priority():
                    nc.vector.tensor_tensor(
                        out=t1b[t][:], in0=ps_hn[t][:, :, 0:ts_],
                        in1=rz16[t][:, 0:2, :], op=ALU.mult,
                    )

            if 0 <= t - 1 < NRT:
                u = t - 1
                us = TS[u]
                # blend, j-split: d_j0 on Pool in parallel with d_j1 on DVE,
                # then e/add on DVE, store via SP — all one iter behind
                d16[u] = work.tile([128, 2, us], bf16, tag=f"d16_{u}", bufs=1,
                                   name=f"d16_{u}")
                e16[u] = work.tile([128, 2, us], bf16, tag=f"e16_{u}", bufs=1,
                                   name=f"e16_{u}")
                ho[u] = work.tile([128, 2, us], bf16, tag=f"ho_{u}", bufs=1,
                                  name=f"ho{u}")
                for j in range(2):
                    deng = nc.gpsimd if j == 0 and u < NRT - 1 else nc.vector
                    deng.tensor_tensor(
                        out=d16[u][:, j, :], in0=h16_t[u][:, j, :],
                        in1=n16[u][:, j, :], op=ALU.subtract,
                    )
                for j in range(2):
                    nc.vector.tensor_tensor(
                        out=e16[u][:, j, :], in0=rz16[u][:, 2 + j, :],
                        in1=d16[u][:, j, :], op=ALU.mult,
                    )
                    nc.vector.tensor_tensor(
                        out=ho[u][:, j, :], in0=n16[u][:, j, :],
                        in1=e16[u][:, j, :], op=ALU.add,
                    )
                nc.sync.dma_start(
                    out=out_d[:, 2 * OFF[u] : 2 * OFF[u + 1]], in_=ho[u][:]
                )

    nc.compile()
    return nc


def _get_nc():
    if "nc" not in _compiled:
        _compiled["nc"] = _build_nc()
    return _compiled["nc"]


def _make_in_maps(h, X_obs, i_obs, W_ih, W_hh, b_ih, b_hh):
    import ml_dtypes

    f = np.float32
    f8 = ml_dtypes.float8_e4m3

    x2 = np.asarray(X_obs, f).reshape(M, IN2)
    hs = np.asarray(h, f)[np.asarray(i_obs)]
    W_ih = np.asarray(W_ih, f)
    W_hh = np.asarray(W_hh, f)
    b_ih = np.asarray(b_ih, f)
    b_hh = np.asarray(b_hh, f)
    brz = b_ih[: 2 * H] + b_hh[: 2 * H]          # [512]
    bin_ = b_ih[2 * H :]                          # [256]
    bhn = b_hh[2 * H :]                           # [256]

    # r/z weight blob: [p, half, 0..3]=pass1 (x,h0), [p, half, 4..7]=pass2
    wrzb = np.zeros((128, 2, 8, 128), f)
    for g in range(4):
        rows = slice(g * 128, (g + 1) * 128)
        wrzb[:, 0, g, :] = W_ih[rows, :].T            # x half
        wrzb[:, 1, g, :] = W_hh[rows, 0:128].T        # h0 half
        wrzb[:, 0, 4 + g, :] = W_hh[rows, 128:256].T  # h1 half
        wrzb[0, 1, 4 + g, :] = brz[rows]              # bias row (pad half)
    # n-path blob: [0..1]=i_n (pad,x), [2..3]=h_n (h0,h1), [4..5]=identity
    wnb = np.zeros((128, 2, 8, 128), f)
    for j in range(2):
        rows = slice(2 * H + j * 128, 2 * H + (j + 1) * 128)
        wnb[0, 0, j, :] = bin_[j * 128 : (j + 1) * 128]  # bias row (pad half)
        wnb[:, 1, j, :] = W_ih[rows, :].T                # x half
        wnb[:, 0, 2 + j, :] = W_hh[rows, 0:128].T
        wnb[:, 1, 2 + j, :] = W_hh[rows, 128:256].T
        wnb[:, j, 4 + j, :] = np.eye(128, dtype=f)       # id for t1_j slot
        wnb[0, 0, 6 + j, :] = bhn[j * 128 : (j + 1) * 128]  # b_hn bias row
    wrzb = wrzb.astype(f8)
    wnb = wnb.astype(f8)

    in_maps = []
    for c in range(NCORES):
        rows0 = c * MC
        rhs8 = np.zeros((128, 5 * MC), f)
        h16 = np.zeros((128, 2 * MC), f)
        for t in range(NRT):
            rs = slice(rows0 + OFF[t], rows0 + OFF[t + 1])
            blk = np.zeros((128, 5, TS[t]), f)
            blk[0, 0, :] = 1.0                    # pad (bias rows)
            blk[:, 1, :] = x2[rs, :].T
            blk[:, 2, :] = hs[rs, 0:128].T
            blk[:, 3, :] = hs[rs, 128:256].T
            blk[0, 4, :] = 1.0                    # pad (bias rows)
            rhs8[:, 5 * OFF[t] : 5 * OFF[t + 1]] = blk.reshape(128, -1)
            hblk = np.stack([hs[rs, 0:128].T, hs[rs, 128:256].T], axis=1)
            h16[:, 2 * OFF[t] : 2 * OFF[t + 1]] = hblk.reshape(128, -1)
        in_maps.append(
            {
                "rhs8": rhs8.astype(f8),
                "h16": h16.astype(ml_dtypes.bfloat16),
                "wrzb": wrzb,
                "wnb": wnb,
            }
        )
    return in_maps


def run_on_device(h, X_obs, i_obs, W_ih, W_hh, b_ih, b_hh, **run_kwargs):
    """Returns (h_new [M,H] fp32, BassKernelResults)."""
    from concourse.bass_utils import run_bass_kernel_spmd

    in_maps = _make_in_maps(h, X_obs, i_obs, W_ih, W_hh, b_ih, b_hh)
    res = run_bass_kernel_spmd(_get_nc(), in_maps, list(range(NCORES)), **run_kwargs)
    parts = []
    for r in res.results:
        ho = np.asarray(r["hout"], dtype=np.float32)      # [128, 2*MC]
        hr = np.zeros((MC, H), np.float32)
        for t in range(NRT):
            blk = ho[:, 2 * OFF[t] : 2 * OFF[t + 1]].reshape(128, 2, TS[t])
            hr[OFF[t] : OFF[t + 1], 0:128] = blk[:, 0, :].T
            hr[OFF[t] : OFF[t + 1], 128:256] = blk[:, 1, :].T
        parts.append(hr)
    h_new = np.concatenate(parts, axis=0)
    return h_new, res


def kernel(h, X_obs, i_obs, W_ih, W_hh, b_ih, b_hh):
    h = np.asarray(h, np.float32)
    i_obs = np.asarray(i_obs)
    h_new, _ = run_on_device(h, X_obs, i_obs, W_ih, W_hh, b_ih, b_hh)
    out = h.copy()
    out[i_obs] = h_new
    return out


# revision 11
# speedup vs baseline: 1.0146x; 1.0052x over previous
"""GRU observation-cell kernel for Trainium2 (8 NeuronCores).

Reference computation (torch GRUCell gate order r,z,n):
    x = X_obs.reshape(M, 128); hs = h[i_obs]
    gi = x@W_ih.T + b_ih; gh = hs@W_hh.T + b_hh
    r = sigmoid(gi_r + gh_r); z = sigmoid(gi_z + gh_z)
    n = tanh(gi_n + b_in + r*(gh_n_raw + b_hn))
    h_new = n + z*(hs - n);  out = h.at[i_obs].set(h_new)

Strategy: data-parallel over the M=20000 observed rows, 2500 rows/core;
host does the (free) gather/scatter, transposes, and dtype casts.  The
device kernel is Act-engine-bound (sigmoid+tanh are Act-only, 768
activation elems/row), so everything else is arranged to hide under the
Act stream:

  - All matmuls are fp8e4 DoubleRow (0.5 PE-cycles per output column,
    256-contract per pass).  rhs tile [128, 5, ts] fp8 with slots
    [pad,x,h0,h1,pad] (pad row0 = 1.0) makes every needed contract pair
    slot-adjacent: i_n=(pad,x), rz1=(x,h0), hn=(h0,h1), rz2=(h1,pad).
    All biases ride pad-row weights, so sigmoid covers r0r1z0z1 in ONE
    Act instruction and tanh needs no bias operand either.
  - PSUM slots are padded to 512 cols so every matmul output is
    bank-aligned: ps_rz [128,4,512] + ps_hn [128,2,512] + ps_n
    [128,2,512] = exactly 8 banks, all bufs=1.
  - n-gate: b_hn is folded into ps_hn via an extra DoubleRow bias pass,
    so t1 = ps_hn * r is ONE plain DVE mult (fp8 out); the next iter PE
    runs i_n (start=True) then an identity DoubleRow pass accumulates t1
    (the zero-weight half ignores the other t1 slot).
  - ~120 tiny warm-up matmuls run while the input DMAs land so the PE is
    at full p-state when tile 0's real matmuls start.
  - Software pipeline (iter t): PE rz/hn-mm(t), in+id-mm(t-1);
    Act sigmoid_rz(t), tanh(t-1); DVE t1(t), blend(t-1); Pool d_j0(t-1);
    SP dma-out(t-1).  Col-tiles [372,500,500,500,500,128]: small edge
    tiles shorten pipeline fill/drain; drain tiles split sigmoid r/z and
    j-halves to shorten the final dependency chains.

Cost-model timing: 28633 ns vs 41682 ns for the fp32r baseline (1.46x);
rel err 7.1e-3 overall (fp8 quantization noise), vs 2e-2 gate.
"""

import numpy as np

N, H, IN2, M, NCORES = 100000, 256, 128, 20000, 8
MC = M // NCORES                      # 2500 observed rows per core
TS = [364, 512, 512, 512, 472, 128]
OFF = np.cumsum([0] + TS).tolist()    # col offsets
NRT = len(TS)

_compiled = {}


def _build_nc():
    from contextlib import ExitStack

    from concourse import bacc
    import concourse.mybir as mybir
    from concourse.tile import TileContext

    dt = mybir.dt
    f32 = dt.float32
    bf16 = dt.bfloat16
    f8 = dt.float8e4
    AF = mybir.ActivationFunctionType
    ALU = mybir.AluOpType
    DR = mybir.MatmulPerfMode.DoubleRow

    nc = bacc.Bacc(None, target_bir_lowering=False)

    # flat per-partition layouts: tile t occupies cols [5*off, 5*(off+ts))
    rhs_d = nc.dram_tensor("rhs8", [128, 5 * MC], f8, kind="ExternalInput")
    wrzb_d = nc.dram_tensor("wrzb", [128, 2, 8, 128], f8, kind="ExternalInput")
    wnb_d = nc.dram_tensor("wnb", [128, 2, 8, 128], f8, kind="ExternalInput")
    h16_d = nc.dram_tensor("h16", [128, 2 * MC], bf16, kind="ExternalInput")
    out_d = nc.dram_tensor("hout", [128, 2 * MC], bf16, kind="ExternalOutput")

    with TileContext(nc) as tc, ExitStack() as ctx:
        const = ctx.enter_context(tc.tile_pool(name="const", bufs=1))
        rhsp = ctx.enter_context(tc.tile_pool(name="rhsp", bufs=1))
        work = ctx.enter_context(tc.tile_pool(name="work", bufs=1))
        psum = ctx.enter_context(tc.tile_pool(name="psum", bufs=1, space="PSUM"))

        # --- PE warm-up: tiny matmuls on a memset tile while DMAs land ---
        warm = const.tile([128, 2, 128], f8, tag="warm")
        nc.gpsimd.memset(warm[:], 0.0)
        ps_warm = psum.tile([128, 4, 512], f32, tag="rz", name="ps_warm")
        for _ in range(112):
            nc.tensor.matmul(
                ps_warm[:, 0, 0:64], lhsT=warm[:, :, :], rhs=warm[:, :, 0:64],
                start=True, stop=True, perf_mode=DR,
            )

        # --- input DMAs (SP queue; rhs tiles all resident) ---
        rhs_t = [None] * NRT
        rhs_t[0] = rhsp.tile([128, 5, TS[0]], f8, tag="rhs0", name="rhs0")
        nc.sync.dma_start(out=rhs_t[0][:], in_=rhs_d[:, 0 : 5 * TS[0]])
        wrzb = const.tile([128, 2, 8, 128], f8, tag="wrzb")
        nc.sync.dma_start(out=wrzb[:], in_=wrzb_d[...])
        wnb = const.tile([128, 2, 8, 128], f8, tag="wnb")
        nc.sync.dma_start(out=wnb[:], in_=wnb_d[...])
        h16_t = [None] * NRT
        h16_t[0] = rhsp.tile([128, 2, TS[0]], bf16, tag="h16_0", name="h16_0")
        nc.sync.dma_start(out=h16_t[0][:], in_=h16_d[:, 0 : 2 * TS[0]])
        for t in range(1, NRT):
            rhs_t[t] = rhsp.tile([128, 5, TS[t]], f8, tag=f"rhs{t}", name=f"rhs{t}")
            nc.sync.dma_start(
                out=rhs_t[t][:], in_=rhs_d[:, 5 * OFF[t] : 5 * OFF[t + 1]]
            )
            h16_t[t] = rhsp.tile([128, 2, TS[t]], bf16, tag=f"h16_{t}",
                                 name=f"h16_{t}")
            nc.sync.dma_start(
                out=h16_t[t][:], in_=h16_d[:, 2 * OFF[t] : 2 * OFF[t + 1]]
            )

        # --- software-pipelined main loop ---
        ps_rz = [None] * NRT
        ps_hn = [None] * NRT
        ps_n = [None] * NRT
        t1b = [None] * NRT
        rz16 = [None] * NRT
        n16 = [None] * NRT
        d16 = [None] * NRT
        e16 = [None] * NRT
        ho = [None] * NRT

        for t in range(NRT + 1):
            if t < NRT:
                r = rhs_t[t]
                ts_ = TS[t]
                # PE: r,z gates — two DoubleRow passes per gate tile
                ps_rz[t] = psum.tile([128, 4, 512], f32, tag="rz", name=f"psrz{t}")
                for g in range(4):
                    nc.tensor.matmul(
                        ps_rz[t][:, g, 0:ts_], lhsT=wrzb[:, :, g, :],
                        rhs=r[:, 1:3, :],
                        start=True, stop=False, perf_mode=DR,
                    )
                    nc.tensor.matmul(
                        ps_rz[t][:, g, 0:ts_], lhsT=wrzb[:, :, 4 + g, :],
                        rhs=r[:, 3:5, :],
                        start=False, stop=True, perf_mode=DR,
                    )
                # PE: h_n gate (contract 256, one DoubleRow pass per half)
                ps_hn[t] = psum.tile([128, 2, 512], f32, tag="hn", name=f"pshn{t}")
                for j in range(2):
                    nc.tensor.matmul(
                        ps_hn[t][:, j, 0:ts_], lhsT=wnb[:, :, 2 + j, :],
                        rhs=r[:, 2:4, :],
                        start=True, stop=False, perf_mode=DR,
                    )
                    nc.tensor.matmul(
                        ps_hn[t][:, j, 0:ts_], lhsT=wnb[:, :, 6 + j, :],
                        rhs=r[:, 0:2, :],
                        start=False, stop=True, perf_mode=DR,
                    )

            if 0 <= t - 1 < NRT:
                u = t - 1
                us = TS[u]
                # PE: i_n (+b_in via pad row), then identity DoubleRow pass
                # adds t1 (the zero-weight half ignores the other t1 slot)
                ps_n[u] = psum.tile([128, 2, 512], f32, tag="n", name=f"psn{u}")
                for j in range(2):
                    nc.tensor.matmul(
                        ps_n[u][:, j, 0:us], lhsT=wnb[:, :, j, :],
                        rhs=rhs_t[u][:, 0:2, :],
                        start=True, stop=False, perf_mode=DR,
                    )
                    nc.tensor.matmul(
                        ps_n[u][:, j, 0:us], lhsT=wnb[:, :, 4 + j, :],
                        rhs=t1b[u][:, 0:2, :],
                        start=False, stop=True, perf_mode=DR,
                    )

            if t < NRT:
                ts_ = TS[t]
                # Act: sigmoid over r/z gate tiles; drain tiles split r first
                # so the t1 chain starts one sigmoid earlier
                rz16[t] = work.tile([128, 4, ts_], bf16, tag=f"rz16_{t}", bufs=1,
                                    name=f"rz16_{t}")
                if t >= NRT - 2:
                    nc.scalar.activation(
                        out=rz16[t][:, 0:2, :], in_=ps_rz[t][:, 0:2, 0:ts_],
                        func=AF.Sigmoid,
                    )
                    nc.scalar.activation(
                        out=rz16[t][:, 2:4, :], in_=ps_rz[t][:, 2:4, 0:ts_],
                        func=AF.Sigmoid,
                    )
                else:
                    nc.scalar.activation(
                        out=rz16[t][:], in_=ps_rz[t][:, :, 0:ts_], func=AF.Sigmoid
                    )

            if 0 <= t - 1 < NRT:
                u = t - 1
                us = TS[u]
                # Act: tanh of previous tile's n gate (software-pipelined)
                n16[u] = work.tile([128, 2, us], bf16, tag=f"n16_{u}", bufs=1,
                                   name=f"n16_{u}")
                if u >= NRT - 2:
                    for j in range(2):
                        nc.scalar.activation(
                            out=n16[u][:, j, :], in_=ps_n[u][:, j, 0:us],
                            func=AF.Tanh,
                        )
                else:
                    nc.scalar.activation(
                        out=n16[u][:], in_=ps_n[u][:, :, 0:us], func=AF.Tanh
                    )

            if t < NRT:
                ts_ = TS[t]
                # DVE: t1 = (ps_hn + b_hn) * r  -> SBUF fp8 (id-matmul rhs)
                t1b[t] = work.tile([128, 2, ts_], f8, tag=f"t1b_{t}", bufs=1,
                                   name=f"t1b{t}")
                with tc.high_priority():
                    nc.vector.tensor_tensor(
                        out=t1b[t][:], in0=ps_hn[t][:, :, 0:ts_],
                        in1=rz16[t][:, 0:2, :], op=ALU.mult,
                    )

            if 0 <= t - 1 < NRT:
                u = t - 1
                us = TS[u]
                # blend, j-split: d_j0 on Pool in parallel with d_j1 on DVE,
                # then e/add on DVE, store via SP — all one iter behind
                d16[u] = work.tile([128, 2, us], bf16, tag=f"d16_{u}", bufs=1,
                                   name=f"d16_{u}")
                e16[u] = work.tile([128, 2, us], bf16, tag=f"e16_{u}", bufs=1,
                                   name=f"e16_{u}")
                ho[u] = work.tile([128, 2, us], bf16, tag=f"ho_{u}", bufs=1,
                                  name=f"ho{u}")
                for j in range(2):
                    deng = nc.gpsimd if j == 0 and u < NRT - 1 else nc.vector
                    deng.tensor_tensor(
                        out=d16[u][:, j, :], in0=h16_t[u][:, j, :],
                        in1=n16[u][:, j, :], op=ALU.subtract,
                    )
                for j in range(2):
                    nc.vector.tensor_tensor(
                        out=e16[u][:, j, :], in0=rz16[u][:, 2 + j, :],
                        in1=d16[u][:, j, :], op=ALU.mult,
                    )
                    nc.vector.tensor_tensor(
                        out=ho[u][:, j, :], in0=n16[u][:, j, :],
                        in1=e16[u][:, j, :], op=ALU.add,
                    )
                nc.sync.dma_start(
                    out=out_d[:, 2 * OFF[u] : 2 * OFF[u + 1]], in_=ho[u][:]
                )

    nc.compile()
    return nc


def _get_nc():
    if "nc" not in _compiled:
        _compiled["nc"] = _build_nc()
    return _compiled["nc"]


def _make_in_maps(h, X_obs, i_obs, W_ih, W_hh, b_ih, b_hh):
    import ml_dtypes

    f = np.float32
    f8 = ml_dtypes.float8_e4m3

    x2 = np.asarray(X_obs, f).reshape(M, IN2)
    hs = np.asarray(h, f)[np.asarray(i_obs)]
    W_ih = np.asarray(W_ih, f)
    W_hh = np.asarray(W_hh, f)
    b_ih = np.asarray(b_ih, f)
    b_hh = np.asarray(b_hh, f)
    brz = b_ih[: 2 * H] + b_hh[: 2 * H]          # [512]
    bin_ = b_ih[2 * H :]                          # [256]
    bhn = b_hh[2 * H :]                           # [256]

    # r/z weight blob: [p, half, 0..3]=pass1 (x,h0), [p, half, 4..7]=pass2
    wrzb = np.zeros((128, 2, 8, 128), f)
    for g in range(4):
        rows = slice(g * 128, (g + 1) * 128)
        wrzb[:, 0, g, :] = W_ih[rows, :].T            # x half
        wrzb[:, 1, g, :] = W_hh[rows, 0:128].T        # h0 half
        wrzb[:, 0, 4 + g, :] = W_hh[rows, 128:256].T  # h1 half
        wrzb[0, 1, 4 + g, :] = brz[rows]              # bias row (pad half)
    # n-path blob: [0..1]=i_n (pad,x), [2..3]=h_n (h0,h1), [4..5]=identity
    wnb = np.zeros((128, 2, 8, 128), f)
    for j in range(2):
        rows = slice(2 * H + j * 128, 2 * H + (j + 1) * 128)
        wnb[0, 0, j, :] = bin_[j * 128 : (j + 1) * 128]  # bias row (pad half)
        wnb[:, 1, j, :] = W_ih[rows, :].T                # x half
        wnb[:, 0, 2 + j, :] = W_hh[rows, 0:128].T
        wnb[:, 1, 2 + j, :] = W_hh[rows, 128:256].T
        wnb[:, j, 4 + j, :] = np.eye(128, dtype=f)       # id for t1_j slot
        wnb[0, 0, 6 + j, :] = bhn[j * 128 : (j + 1) * 128]  # b_hn bias row
    wrzb = wrzb.astype(f8)
    wnb = wnb.astype(f8)

    in_maps = []
    for c in range(NCORES):
        rows0 = c * MC
        rhs8 = np.zeros((128, 5 * MC), f)
        h16 = np.zeros((128, 2 * MC), f)
        for t in range(NRT):
            rs = slice(rows0 + OFF[t], rows0 + OFF[t + 1])
            blk = np.zeros((128, 5, TS[t]), f)
            blk[0, 0, :] = 1.0                    # pad (bias rows)
            blk[:, 1, :] = x2[rs, :].T
            blk[:, 2, :] = hs[rs, 0:128].T
            blk[:, 3, :] = hs[rs, 128:256].T
            blk[0, 4, :] = 1.0                    # pad (bias rows)
            rhs8[:, 5 * OFF[t] : 5 * OFF[t + 1]] = blk.reshape(128, -1)
            hblk = np.stack([hs[rs, 0:128].T, hs[rs, 128:256].T], axis=1)
            h16[:, 2 * OFF[t] : 2 * OFF[t + 1]] = hblk.reshape(128, -1)
        in_maps.append(
            {
                "rhs8": rhs8.astype(f8),
                "h16": h16.astype(ml_dtypes.bfloat16),
                "wrzb": wrzb,
                "wnb": wnb,
            }
        )
    return in_maps


def run_on_device(h, X_obs, i_obs, W_ih, W_hh, b_ih, b_hh, **run_kwargs):
    """Returns (h_new [M,H] fp32, BassKernelResults)."""
    from concourse.bass_utils import run_bass_kernel_spmd

    in_maps = _make_in_maps(h, X_obs, i_obs, W_ih, W_hh, b_ih, b_hh)
    res = run_bass_kernel_spmd(_get_nc(), in_maps, list(range(NCORES)), **run_kwargs)
    parts = []
    for r in res.results:
        ho = np.asarray(r["hout"], dtype=np.float32)      # [128, 2*MC]
        hr = np.zeros((MC, H), np.float32)
        for t in range(NRT):
            blk = ho[:, 2 * OFF[t] : 2 * OFF[t + 1]].reshape(128, 2, TS[t])
            hr[OFF[t] : OFF[t + 1], 0:128] = blk[:, 0, :].T
            hr[OFF[t] : OFF[t + 1], 128:256] = blk[:, 1, :].T
        parts.append(hr)
    h_new = np.concatenate(parts, axis=0)
    return h_new, res


def kernel(h, X_obs, i_obs, W_ih, W_hh, b_ih, b_hh):
    h = np.asarray(h, np.float32)
    i_obs = np.asarray(i_obs)
    h_new, _ = run_on_device(h, X_obs, i_obs, W_ih, W_hh, b_ih, b_hh)
    out = h.copy()
    out[i_obs] = h_new
    return out
